# revision 4
# baseline (speedup 1.0000x reference)
"""Trainium2 Bass kernel for nn_GameTensor_27195732918735.

Computes out[i,j,b] = Hessian_z V_i(z_all[j,b]) for i != j, zeros on the
diagonal, where V_i(z) = W2[i] @ tanh(W1[i] @ z + b1[i]) + b2[i].

Analytic form used on-device:
    u = W1 z + b1;  th = tanh(u);  s_k = -2 W2_k th_k (1 - th_k^2)
    H = W1^T diag(s) W1  =  sum_k s_k w1_k w1_k^T

Per-core plan (8 cores, SPMD):
  core c owns agent i = c//2 and three (j, batch-half) "tasks" (the 12
  nonzero (i,j) cells x 2 batch halves = 24 half-cells / 8 cores = 3).
  On-chip: T[k, a*128+c] = W1[k,a] * W1[k,c] is precomputed once per core
  (agent-local), then each task's Hessians for its 128 batches are a single
  [k=256] x [b=128] x [(a,c)=16384] matmul H[b,(a,c)] = sum_k S[k,b] T[k,(a,c)]
  with perfectly contiguous output DMA. Diagonal zero blocks are written
  host-side (they are constants).
"""

import numpy as np

import concourse.bass as bass
import concourse.mybir as mybir
import concourse.tile as tile
from concourse import bacc
from concourse.bass_utils import run_bass_kernel_spmd

N, B, D = 4, 256, 128
H2 = 2 * D  # 256 hidden
NCORES = 8
NTASK = 3  # (j, half) tasks per core
HALF = B // 2  # 128 batches per task

# matmul operand dtype for the big S^T @ T matmuls:
#   "bf16"  : bfloat16 operands (1 cyc/row, ~0.3% rel err)
#   "f32r"  : float32r operands (4-byte, 1 cyc/row at N>=512 per cost model)
#   "f32"   : plain float32 (4 cyc/row, exact)
MM_MODE = "f32r"

_F32 = mybir.dt.float32


def _mm_store_dtype():
    if MM_MODE == "bf16":
        return mybir.dt.bfloat16
    if MM_MODE == "f32r":
        return mybir.dt.float32r
    return _F32


def _mm_view(ap):
    return ap


def _emit(tc, nc, w1c, w1t, b1c, w2s, zt, out):
    mmdt = _mm_store_dtype()
    Tanh = mybir.ActivationFunctionType.Tanh
    mult = mybir.AluOpType.mult
    add = mybir.AluOpType.add

    with (
        tc.tile_pool(name="consts", bufs=1) as consts,
        tc.tile_pool(name="tpool", bufs=1) as tpool,
        tc.tile_pool(name="small", bufs=4) as small,
        tc.tile_pool(name="stage", bufs=3) as stage_pool,
        tc.tile_pool(name="upsum", bufs=2, space="PSUM") as upsum,
        tc.tile_pool(name="psum", bufs=6, space="PSUM") as psum,
    ):
        # ---- load constants -------------------------------------------------
        w1c_sb = consts.tile([128, 2, 128], _F32)  # [k%128, kchunk, c]
        nc.sync.dma_start(w1c_sb, w1c)
        w1t_sb = consts.tile([128, 256], _F32)  # [d, k]
        nc.sync.dma_start(w1t_sb, w1t)
        b1_sb = consts.tile([128, 2], _F32)
        nc.sync.dma_start(b1_sb, b1c)
        w2s_sb = consts.tile([128, 2], _F32)  # -2*W2, [k%128, kchunk]
        nc.sync.dma_start(w2s_sb, w2s)
        zt_sb = consts.tile([128, NTASK, 128], _F32)  # [d, task, b]
        nc.sync.dma_start(zt_sb, zt.rearrange("t d b -> d t b"))

        if mmdt == mybir.dt.bfloat16:
            w1m = consts.tile([128, 2, 128], mmdt)
            nc.vector.tensor_copy(out=w1m, in_=w1c_sb)
        else:
            w1m = w1c_sb

        # ---- S[k, b] per task: s = -2*W2 * th * (1 - th^2) ------------------
        s_sb = consts.tile([128, NTASK, 2, 128], mmdt)  # [k%128, task, kchunk, b]
        for t in range(NTASK):
            for kc in range(2):
                ups = upsum.tile([128, 128], _F32)
                nc.tensor.matmul(
                    ups,
                    lhsT=w1t_sb[:, kc * 128 : (kc + 1) * 128],
                    rhs=zt_sb[:, t, :],
                    start=True,
                    stop=True,
                )
                th = small.tile([128, 128], _F32, tag="th")
                nc.scalar.activation(th, ups, Tanh, bias=b1_sb[:, kc : kc + 1])
                sq = small.tile([128, 128], _F32, tag="sq")
                nc.vector.tensor_tensor(sq, th, th, mult)
                nc.vector.tensor_scalar(sq, sq, -1.0, 1.0, mult, add)
                nc.vector.tensor_tensor(sq, th, sq, mult)
                nc.vector.tensor_scalar(
                    s_sb[:, t, kc, :], sq, w2s_sb[:, kc : kc + 1], None, mult
                )

        # ---- T[k, a*128+c] = W1[k,a] * W1[k,c], 8 a-values per DVE op -------
        AG = 8  # a-values per op
        TT = tpool.tile([128, 2, 16384], mmdt)
        for g in range(128 // AG):
            for kc in range(2):
                dst = TT[:, kc, g * AG * 128 : (g + 1) * AG * 128].rearrange(
                    "p (x y) -> p x y", x=AG
                )
                in0 = w1m[:, kc, None, :].to_broadcast((128, AG, 128))
                in1 = w1m[:, kc, g * AG : (g + 1) * AG, None].to_broadcast(
                    (128, AG, 128)
                )
                nc.vector.tensor_tensor(dst, in0, in1, mult)

        # ---- main: H[b, (a,c)] = sum_k S[k,b] T[k,(a,c)] --------------------
        out_flat = [out[t].rearrange("b a c -> b (a c)") for t in range(NTASK)]
        for t in range(NTASK):
            for g4 in range(8):  # 4 n-tiles of 512 -> one 1 MiB DMA
                stg = stage_pool.tile([128, 2048], _F32)
                for nn in range(4):
                    n = g4 * 4 + nn
                    ps = psum.tile([128, 512], _F32)
                    nc.tensor.matmul(
                        ps,
                        lhsT=_mm_view(s_sb[:, t, 0, :]),
                        rhs=_mm_view(TT[:, 0, n * 512 : (n + 1) * 512]),
                        start=True,
                        stop=False,
                    )
                    nc.tensor.matmul(
                        ps,
                        lhsT=_mm_view(s_sb[:, t, 1, :]),
                        rhs=_mm_view(TT[:, 1, n * 512 : (n + 1) * 512]),
                        start=False,
                        stop=True,
                    )
                    dst = stg[:, nn * 512 : (nn + 1) * 512]
                    if n % 3 == 2:
                        nc.scalar.copy(dst, ps)
                    else:
                        nc.vector.tensor_copy(out=dst, in_=ps)
                nc.sync.dma_start(out_flat[t][:, g4 * 2048 : (g4 + 1) * 2048], stg)


_NC_CACHE = {}


def _core_tasks(c):
    i = c // 2
    js = [j for j in range(N) if j != i]
    halves = [(j, h) for j in js for h in (0, 1)]
    return i, (halves[0:3] if c % 2 == 0 else halves[3:6])


def _build():
    key = MM_MODE
    if key in _NC_CACHE:
        return _NC_CACHE[key]
    nc = bacc.Bacc("TRN2", target_bir_lowering=False, debug=False, num_devices=NCORES)
    w1c = nc.dram_tensor("w1c", [128, 2, 128], _F32, kind="ExternalInput").ap()
    w1t = nc.dram_tensor("w1t", [128, 256], _F32, kind="ExternalInput").ap()
    b1c = nc.dram_tensor("b1c", [128, 2], _F32, kind="ExternalInput").ap()
    w2s = nc.dram_tensor("w2s", [128, 2], _F32, kind="ExternalInput").ap()
    zt = nc.dram_tensor("zt", [NTASK, 128, 128], _F32, kind="ExternalInput").ap()
    out = nc.dram_tensor("out", [NTASK, HALF, D, D], _F32, kind="ExternalOutput").ap()
    with tile.TileContext(nc) as tc:
        _emit(tc, nc, w1c, w1t, b1c, w2s, zt, out)
    nc.compile()
    _NC_CACHE[key] = nc
    return nc


# Options for test harness introspection (set by test.py, unused in grading).
_RUN_KWARGS = {}
_LAST_RESULT = None


def kernel(z_all, W1, b1, W2, b2):
    global _LAST_RESULT
    z_all = np.asarray(z_all, dtype=np.float32)
    W1 = np.asarray(W1, dtype=np.float32)
    b1 = np.asarray(b1, dtype=np.float32)
    W2 = np.asarray(W2, dtype=np.float32)

    nc = _build()

    in_maps = []
    metas = []
    for c in range(NCORES):
        i, tasks = _core_tasks(c)
        metas.append((i, tasks))
        w1i = W1[i]  # [256, 128]
        in_maps.append(
            {
                "w1c": np.ascontiguousarray(
                    w1i.reshape(2, 128, 128).transpose(1, 0, 2)
                ),
                "w1t": np.ascontiguousarray(w1i.T),
                "b1c": np.ascontiguousarray(b1[i].reshape(2, 128).T),
                "w2s": np.ascontiguousarray((-2.0 * W2[i, 0]).reshape(2, 128).T),
                "zt": np.ascontiguousarray(
                    np.stack(
                        [
                            z_all[j, h * HALF : (h + 1) * HALF, :].T
                            for (j, h) in tasks
                        ]
                    )
                ),
            }
        )

    res = run_bass_kernel_spmd(nc, in_maps, list(range(NCORES)), **_RUN_KWARGS)
    _LAST_RESULT = res

    full = np.zeros((N, N, B, D, D), dtype=np.float32)
    for c in range(NCORES):
        i, tasks = metas[c]
        o = res.results[c]["out"]  # [NTASK, HALF, D, D]
        for t, (j, h) in enumerate(tasks):
            full[i, j, h * HALF : (h + 1) * HALF] = o[t]
    return full
